# revision 32
# baseline (speedup 1.0000x reference)
"""ContrastiveCenterLoss on 8 Trainium2 NeuronCores.

Math: with dist[b,c] = ||f_b - c_c||^2,
  intra = sum_b dist[b, label_b]          = sum_b ||f_b - c_{label_b}||^2
  total = sum_{b,c} dist[b,c]             = C*sum||f||^2 + B*sum||c||^2 - 2*(sum_b f_b)@(sum_c c_c)
  inter = total - intra
  loss  = (1/2/B) * intra / (inter + 1e-6) / 0.1

Sharding: feat/label batch-sharded (2048 rows/core); centers statistics
sharded over 512-row slices; the full centers table stays in HBM and is
row-gathered by label via multi-index indirect DMA (bf16 cast on the way
in).  Per core the kernel emits a single [128, 16] fp32 stats tile:
per-partition partial sums of cs^2, f^2, (f-g)^2 plus the per-d column
sums F = sum_b f_b and Cs = sum_c c_c (from [128,1]-output PE matmuls).
Host all-reduces the per-core partials in float64 and applies the final
scalar division.
"""

import numpy as np

B, C, D = 16384, 4096, 128
LAMBDA_C = 1.0
NCORES = 8
BS = B // NCORES          # 2048 feat rows per core
NPT = BS // 128           # 16 feat rows per partition
CS = C // NCORES          # 512 center rows per core (stats slice)
CSPT = CS // 128          # 4 center rows per partition

# tuned via TimelineSim search (see test.py / search harness)
CFG = dict(
    chunks=(10, 16),      # gather chunk boundaries in slot units
    # per chunk: list of (sub_engine, d2_engine, nslots) pieces.
    # sub_engine: 'dve' (TT bf16 2x) or 'pool' (gpsimd TT).
    # d2_engine: 'dve' (stt accum) or 'act' (Square accum).
    # tuned via TimelineSim search (search.py / search3.py)
    pieces=(
        (('dve', 'act', 5), ('dve', 'act', 5)),
        (('dve', 'dve', 6),),
    ),
    f2_dve_slots=4,       # slots of f^2 on DVE (rest ACT)
    f2_act_slots=12,
    labt_engine='sync',   # 'sync' (HWDGE) or 'gpsimd' (SWDGE)
    cs2_engine='dve',     # 'dve' | 'act'
    f_t_split=8,          # slots in first feat DMA (0 = single DMA)
    idx_from='sbuf',      # 'sbuf' (DMA labels in first) or 'dram' (direct)
)

_cached = {}


def _build_nc(repeat=1, cfg=None):
    import concourse.bass as bass
    import concourse.tile as tile
    from concourse import bacc, mybir

    if cfg is None:
        cfg = CFG
    chunks = cfg['chunks']
    pieces = cfg['pieces']
    sizes = [b - a for a, b in zip((0,) + tuple(chunks[:-1]), chunks)]
    f2_dve_slots = cfg['f2_dve_slots']
    f2_act_slots = cfg.get('f2_act_slots', NPT - cfg['f2_dve_slots'])
    labt_engine = cfg['labt_engine']
    cs2_engine = cfg['cs2_engine']
    idx_from = cfg.get('idx_from', 'sbuf')
    f_t_split = cfg.get('f_t_split', 0)   # slots in first feat DMA (0=one)
    assert len(pieces) == len(chunks)
    for s, pl in zip(sizes, pieces):
        assert sum(p[2] for p in pl) == s, (sizes, pieces)
        # 'dve'/'act' accumulate directly; 'actpe'/'poolpe' square without
        # an accumulator and reduce via [128,1]-output PE matmuls
        assert all(p[1] in ('dve', 'act', 'actpe', 'poolpe') for p in pl)
        assert all(p[0] in ('dve', 'pool') for p in pl)
    n_pe = sum(p[2] for pl in pieces for p in pl if p[1] in ('actpe', 'poolpe'))
    n_pieces = sum(len(pl) for pl in pieces)
    assert n_pieces <= 9, "not enough stats columns (3..11 for d^2)"
    assert f2_dve_slots + f2_act_slots <= NPT

    f32 = mybir.dt.float32
    bf16 = mybir.dt.bfloat16
    i32 = mybir.dt.int32
    Alu = mybir.AluOpType
    Act = mybir.ActivationFunctionType

    nc = bacc.Bacc("TRN2", target_bir_lowering=False, debug=False,
                   num_devices=NCORES)

    feat = nc.dram_tensor("feat", [BS, D], f32, kind="ExternalInput")
    labt = nc.dram_tensor("labt", [128, NPT], i32, kind="ExternalInput")
    centers = nc.dram_tensor("centers", [C, D], f32, kind="ExternalInput")
    cslice = nc.dram_tensor("cslice", [CS, D], f32, kind="ExternalInput")

    o_stats = nc.dram_tensor("o_stats", [128, 16], f32, kind="ExternalOutput")

    # stats column map; host sums cols 3..13 as intra, 1..2 (+13) as f^2
    COL_CS2 = 0
    COL_F2 = 1            # 1 dve, 2 act
    COL_D2 = 3            # 3 + chunk (DVE), 6 + chunk (ACT), 9 + chunk (Pool)
    COL_D2A = 6
    COL_D2P = 9
    COL_F2P = 12          # f^2 pool part
    COL_F = 14
    COL_C = 15

    # partition-major views: partition p holds rows p*NPT .. p*NPT+NPT-1
    fv = feat.ap().rearrange("(p n) d -> p n d", p=128)
    csv = cslice.ap().rearrange("(p n) d -> p n d", p=128)

    bounds = [0] + list(chunks)
    nch = len(chunks)

    with tile.TileContext(nc) as tc:
        with tc.tile_pool(name="const", bufs=1) as cpool, \
             tc.tile_pool(name="sbuf", bufs=2) as pool, \
             tc.tile_pool(name="psum", bufs=2, space="PSUM") as psum:

            ones_bf = cpool.tile([128, 1], bf16)
            nc.vector.memset(ones_bf[:], 1.0)
            ones_f32 = cpool.tile([128, 1], f32)
            nc.vector.memset(ones_f32[:], 1.0)
            # warm the ACT Square table during the load phase: the table
            # load costs ~1.3us and otherwise lands right before the first
            # real Square, stalling the ACT stream mid-kernel.
            warm = cpool.tile([128, 1], f32)
            nc.scalar.activation(
                out=warm[:], in_=ones_f32[:],
                func=mybir.ActivationFunctionType.Square)

            for _ in range(repeat):
                lab = pool.tile([128, NPT], i32, tag="lab")
                f_t = pool.tile([128, NPT * D], bf16, tag="f_t")
                cg = pool.tile([128, NPT * D], bf16, tag="cg")
                d_t = pool.tile([128, NPT * D], bf16, tag="d_t")
                cs_t = pool.tile([128, CSPT * D], f32, tag="cs_t")
                cs_scr = pool.tile([128, CSPT * D], f32, tag="cs_scr")
                f_scr = pool.tile([128, NPT * D], bf16, tag="f_scr")
                stats = pool.tile([128, 16], f32, tag="stats")

                ps_f = psum.tile([128, 1], f32, tag="ps_f")
                ps_c = psum.tile([128, 1], f32, tag="ps_c")
                if n_pe:
                    ps_d = psum.tile([128, 1], f32, tag="ps_d")

                # ---- loads ----
                if idx_from == 'sbuf':
                    if labt_engine == 'sync':
                        nc.sync.dma_start(out=lab[:], in_=labt.ap())
                    else:
                        nc.gpsimd.dma_start(out=lab[:], in_=labt.ap())
                # Pool/SWDGE: feat with fp32->bf16 cast on the fly,
                # optionally in two pieces so compute can start earlier
                # (and the cs transfer can slot between them).
                if f_t_split > 0:
                    nc.gpsimd.dma_start(out=f_t[:, :f_t_split * D],
                                        in_=fv[:, :f_t_split, :])
                    nc.gpsimd.dma_start(out=f_t[:, f_t_split * D:],
                                        in_=fv[:, f_t_split:, :])
                else:
                    nc.gpsimd.dma_start(out=f_t[:], in_=fv[:, :, :])
                # HWDGE: center-slice (fp32)
                nc.sync.dma_start(out=cs_t[:], in_=csv[:, :, :])

                # Pool/SWDGE: one multi-index gather per chunk, bf16 out.
                # Index array read straight from DRAM in 'dram' mode: the
                # ExternalInput is staged in HBM before kernel start, so the
                # gather emission has no DMA dependency to wait on.
                for c in range(nch):
                    s0, s1 = bounds[c], bounds[c + 1]
                    idx_ap = (labt.ap()[:, s0:s1] if idx_from == 'dram'
                              else lab[:, s0:s1])
                    nc.gpsimd.indirect_dma_start(
                        out=cg[:, s0 * D:s1 * D],
                        out_offset=None,
                        in_=centers.ap(),
                        in_offset=bass.IndirectOffsetOnAxis(
                            ap=idx_ap, axis=0),
                    )

                # ---- elementwise reductions (3-way DVE/ACT/Pool split) ----
                # zero unused stats columns before any accum lands
                nc.vector.memset(stats[:], 0.0)

                # f^2: DVE part first (feat lands before cs in the DMA
                # queue), then ACT part split at the f_t_split boundary so
                # ACT can start on the first feat half early.
                if f2_dve_slots > 0:
                    sl = slice(0, f2_dve_slots * D)
                    nc.vector.scalar_tensor_tensor(
                        out=f_scr[:, sl], in0=f_t[:, sl], scalar=1.0,
                        in1=f_t[:, sl], op0=Alu.mult, op1=Alu.mult,
                        accum_out=stats[:, COL_F2:COL_F2 + 1])
                a0, a1 = f2_dve_slots, f2_dve_slots + f2_act_slots
                amid = min(max(f_t_split, a0), a1) if f_t_split > 0 else a1
                if amid > a0:
                    sl = slice(a0 * D, amid * D)
                    nc.scalar.activation(
                        out=f_scr[:, sl], in_=f_t[:, sl], func=Act.Square,
                        accum_out=stats[:, COL_F2 + 1:COL_F2 + 2])
                if a1 > amid:
                    sl = slice(amid * D, a1 * D)
                    nc.scalar.activation(
                        out=f_scr[:, sl], in_=f_t[:, sl], func=Act.Square,
                        accum_out=stats[:, COL_F2P:COL_F2P + 1])

                if cs2_engine == 'dve':
                    nc.vector.scalar_tensor_tensor(
                        out=cs_scr[:], in0=cs_t[:], scalar=1.0, in1=cs_t[:],
                        op0=Alu.mult, op1=Alu.mult,
                        accum_out=stats[:, COL_CS2:COL_CS2 + 1])
                else:
                    nc.scalar.activation(
                        out=cs_scr[:], in_=cs_t[:], func=Act.Square,
                        accum_out=stats[:, COL_CS2:COL_CS2 + 1])

                # per-chunk, per-piece: d = f - g (TT), then d^2 accum.
                # Each piece gets its own stats column so the dependency
                # graph stays piece-local.
                col = COL_D2
                pe_ranges = []
                for c in range(nch):
                    s0 = bounds[c]
                    # subs first (in piece order), then the d^2 ops, so an
                    # engine's d^2 for piece i never blocks a later sub.
                    ranges = []
                    p0 = s0
                    for sub_e, d2_e, ns in pieces[c]:
                        sl = slice(p0 * D, (p0 + ns) * D)
                        ranges.append((sl, d2_e))
                        eng = nc.vector if sub_e == 'dve' else nc.gpsimd
                        eng.tensor_tensor(
                            out=d_t[:, sl], in0=f_t[:, sl], in1=cg[:, sl],
                            op=Alu.subtract)
                        p0 += ns
                    for sl, d2_e in ranges:
                        if d2_e == 'dve':
                            nc.vector.scalar_tensor_tensor(
                                out=d_t[:, sl], in0=d_t[:, sl], scalar=1.0,
                                in1=d_t[:, sl], op0=Alu.mult, op1=Alu.mult,
                                accum_out=stats[:, col:col + 1])
                            col += 1
                        elif d2_e == 'act':
                            nc.scalar.activation(
                                out=cg[:, sl], in_=d_t[:, sl],
                                func=Act.Square,
                                accum_out=stats[:, col:col + 1])
                            col += 1
                        elif d2_e == 'actpe':
                            nc.scalar.activation(
                                out=cg[:, sl], in_=d_t[:, sl],
                                func=Act.Square)
                            pe_ranges.append(sl)
                        else:  # poolpe
                            nc.gpsimd.tensor_tensor(
                                out=cg[:, sl], in0=d_t[:, sl],
                                in1=d_t[:, sl], op=Alu.mult)
                            pe_ranges.append(sl)

                # ---- PE: column sums via [128,1]-output matmuls ----
                for j in range(CSPT):
                    nc.tensor.matmul(out=ps_c[:], lhsT=cs_t[:, j * D:(j + 1) * D],
                                     rhs=ones_f32[:], start=(j == 0),
                                     stop=(j == CSPT - 1))
                for j in range(NPT):
                    nc.tensor.matmul(out=ps_f[:], lhsT=f_t[:, j * D:(j + 1) * D],
                                     rhs=ones_bf[:], start=(j == 0),
                                     stop=(j == NPT - 1))
                # d^2 pe-reduced pieces: one matmul per 128-col slot into a
                # shared psum accumulate chain, then one copy to stats
                if pe_ranges:
                    blocks = []
                    for sl in pe_ranges:
                        for b in range(sl.start, sl.stop, D):
                            blocks.append(b)
                    for i, b in enumerate(blocks):
                        nc.tensor.matmul(out=ps_d[:], lhsT=cg[:, b:b + D],
                                         rhs=ones_bf[:], start=(i == 0),
                                         stop=(i == len(blocks) - 1))
                    nc.scalar.copy(stats[:, 13:14], ps_d[:])

                nc.scalar.copy(stats[:, COL_C:COL_C + 1], ps_c[:])
                nc.scalar.copy(stats[:, COL_F:COL_F + 1], ps_f[:])

                # ---- single output DMA ----
                nc.sync.dma_start(out=o_stats.ap(), in_=stats[:])

    nc.compile()
    return nc


def _get_nc(repeat=1, cfg=None):
    if cfg is None:
        cfg = CFG
    key = ("nc3", repeat, tuple(sorted(
        (k, tuple(v) if isinstance(v, (list, tuple)) else v)
        for k, v in cfg.items())))
    if key not in _cached:
        _cached[key] = _build_nc(repeat, cfg)
    return _cached[key]


def _make_in_maps(feat, label, centers):
    feat = np.ascontiguousarray(np.asarray(feat, dtype=np.float32))
    centers = np.ascontiguousarray(np.asarray(centers, dtype=np.float32))
    lab = np.asarray(label).astype(np.int32)
    in_maps = []
    for k in range(NCORES):
        fs = feat[k * BS:(k + 1) * BS]
        ls = lab[k * BS:(k + 1) * BS].reshape(128, NPT)
        cs = centers[k * CS:(k + 1) * CS]
        in_maps.append({
            "feat": np.ascontiguousarray(fs),
            "labt": np.ascontiguousarray(ls),
            "centers": centers,
            "cslice": np.ascontiguousarray(cs),
        })
    return in_maps


def _combine(results):
    sum_cs2 = 0.0
    sum_f2 = 0.0
    intra = 0.0
    F = np.zeros(D, dtype=np.float64)
    Cv = np.zeros(D, dtype=np.float64)
    for r in results:
        s = r["o_stats"].astype(np.float64)
        sum_cs2 += s[:, 0].sum()
        sum_f2 += s[:, 1:3].sum() + s[:, 12].sum()
        intra += s[:, 3:12].sum() + s[:, 13].sum()
        F += s[:, 14]
        Cv += s[:, 15]
    total = C * sum_f2 + B * sum_cs2 - 2.0 * float(F @ Cv)
    inter = total - intra
    loss = (LAMBDA_C / 2.0 / B) * intra / (inter + 1e-6) / 0.1
    return np.float32(loss)


def kernel(feat, label, centers):
    from concourse.bass_utils import run_bass_kernel_spmd

    nc = _get_nc()
    in_maps = _make_in_maps(feat, label, centers)
    res = run_bass_kernel_spmd(nc, in_maps, list(range(NCORES)))
    return _combine(res.results)


# revision 33
# speedup vs baseline: 1.0014x; 1.0014x over previous
"""ContrastiveCenterLoss on 8 Trainium2 NeuronCores.

Math: with dist[b,c] = ||f_b - c_c||^2,
  intra = sum_b dist[b, label_b]          = sum_b ||f_b - c_{label_b}||^2
  total = sum_{b,c} dist[b,c]             = C*sum||f||^2 + B*sum||c||^2 - 2*(sum_b f_b)@(sum_c c_c)
  inter = total - intra
  loss  = (1/2/B) * intra / (inter + 1e-6) / 0.1

Sharding: feat/label batch-sharded (2048 rows/core); centers statistics
sharded over 512-row slices; the full centers table stays in HBM and is
row-gathered by label via multi-index indirect DMA (bf16 cast on the way
in).  Per core the kernel emits a single [128, 16] fp32 stats tile:
per-partition partial sums of cs^2, f^2, (f-g)^2 plus the per-d column
sums F = sum_b f_b and Cs = sum_c c_c (from [128,1]-output PE matmuls).
Host all-reduces the per-core partials in float64 and applies the final
scalar division.
"""

import numpy as np

B, C, D = 16384, 4096, 128
LAMBDA_C = 1.0
NCORES = 8
BS = B // NCORES          # 2048 feat rows per core
NPT = BS // 128           # 16 feat rows per partition
CS = C // NCORES          # 512 center rows per core (stats slice)
CSPT = CS // 128          # 4 center rows per partition

# tuned via TimelineSim search (see test.py / search harness)
CFG = dict(
    chunks=(10, 16),      # gather chunk boundaries in slot units
    # per chunk: list of (sub_engine, d2_engine, nslots) pieces.
    # sub_engine: 'dve' (TT bf16 2x) or 'pool' (gpsimd TT).
    # d2_engine: 'dve' (stt accum) or 'act' (Square accum).
    # tuned via TimelineSim search (search.py / search3.py / search4.py)
    pieces=(
        (('dve', 'actpe', 5), ('dve', 'act', 5)),
        (('dve', 'poolpe', 3), ('dve', 'dve', 3)),
    ),
    f2_dve_slots=4,       # slots of f^2 on DVE (rest ACT)
    f2_act_slots=12,
    labt_engine='sync',   # 'sync' (HWDGE) or 'gpsimd' (SWDGE)
    cs2_engine='dve',     # 'dve' | 'act'
    f_t_split=8,          # slots in first feat DMA (0 = single DMA)
    idx_from='sbuf',      # 'sbuf' (DMA labels in first) or 'dram' (direct)
)

_cached = {}


def _build_nc(repeat=1, cfg=None):
    import concourse.bass as bass
    import concourse.tile as tile
    from concourse import bacc, mybir

    if cfg is None:
        cfg = CFG
    chunks = cfg['chunks']
    pieces = cfg['pieces']
    sizes = [b - a for a, b in zip((0,) + tuple(chunks[:-1]), chunks)]
    f2_dve_slots = cfg['f2_dve_slots']
    f2_act_slots = cfg.get('f2_act_slots', NPT - cfg['f2_dve_slots'])
    labt_engine = cfg['labt_engine']
    cs2_engine = cfg['cs2_engine']
    idx_from = cfg.get('idx_from', 'sbuf')
    f_t_split = cfg.get('f_t_split', 0)   # slots in first feat DMA (0=one)
    assert len(pieces) == len(chunks)
    for s, pl in zip(sizes, pieces):
        assert sum(p[2] for p in pl) == s, (sizes, pieces)
        # 'dve'/'act' accumulate directly; 'actpe'/'poolpe' square without
        # an accumulator and reduce via [128,1]-output PE matmuls
        assert all(p[1] in ('dve', 'act', 'actpe', 'poolpe') for p in pl)
        assert all(p[0] in ('dve', 'pool') for p in pl)
    n_pe = sum(p[2] for pl in pieces for p in pl if p[1] in ('actpe', 'poolpe'))
    n_pieces = sum(len(pl) for pl in pieces)
    assert n_pieces <= 9, "not enough stats columns (3..11 for d^2)"
    assert f2_dve_slots + f2_act_slots <= NPT

    f32 = mybir.dt.float32
    bf16 = mybir.dt.bfloat16
    i32 = mybir.dt.int32
    Alu = mybir.AluOpType
    Act = mybir.ActivationFunctionType

    nc = bacc.Bacc("TRN2", target_bir_lowering=False, debug=False,
                   num_devices=NCORES)

    feat = nc.dram_tensor("feat", [BS, D], f32, kind="ExternalInput")
    labt = nc.dram_tensor("labt", [128, NPT], i32, kind="ExternalInput")
    centers = nc.dram_tensor("centers", [C, D], f32, kind="ExternalInput")
    cslice = nc.dram_tensor("cslice", [CS, D], f32, kind="ExternalInput")

    o_stats = nc.dram_tensor("o_stats", [128, 16], f32, kind="ExternalOutput")

    # stats column map; host sums cols 3..13 as intra, 1..2 (+13) as f^2
    COL_CS2 = 0
    COL_F2 = 1            # 1 dve, 2 act
    COL_D2 = 3            # 3 + chunk (DVE), 6 + chunk (ACT), 9 + chunk (Pool)
    COL_D2A = 6
    COL_D2P = 9
    COL_F2P = 12          # f^2 pool part
    COL_F = 14
    COL_C = 15

    # partition-major views: partition p holds rows p*NPT .. p*NPT+NPT-1
    fv = feat.ap().rearrange("(p n) d -> p n d", p=128)
    csv = cslice.ap().rearrange("(p n) d -> p n d", p=128)

    bounds = [0] + list(chunks)
    nch = len(chunks)

    with tile.TileContext(nc) as tc:
        with tc.tile_pool(name="const", bufs=1) as cpool, \
             tc.tile_pool(name="sbuf", bufs=2) as pool, \
             tc.tile_pool(name="psum", bufs=2, space="PSUM") as psum:

            ones_bf = cpool.tile([128, 1], bf16)
            nc.vector.memset(ones_bf[:], 1.0)
            ones_f32 = cpool.tile([128, 1], f32)
            nc.vector.memset(ones_f32[:], 1.0)
            # warm the ACT Square table during the load phase: the table
            # load costs ~1.3us and otherwise lands right before the first
            # real Square, stalling the ACT stream mid-kernel.
            warm = cpool.tile([128, 1], f32)
            nc.scalar.activation(
                out=warm[:], in_=ones_f32[:],
                func=mybir.ActivationFunctionType.Square)

            for _ in range(repeat):
                lab = pool.tile([128, NPT], i32, tag="lab")
                f_t = pool.tile([128, NPT * D], bf16, tag="f_t")
                cg = pool.tile([128, NPT * D], bf16, tag="cg")
                d_t = pool.tile([128, NPT * D], bf16, tag="d_t")
                cs_t = pool.tile([128, CSPT * D], f32, tag="cs_t")
                cs_scr = pool.tile([128, CSPT * D], f32, tag="cs_scr")
                f_scr = pool.tile([128, NPT * D], bf16, tag="f_scr")
                stats = pool.tile([128, 16], f32, tag="stats")

                ps_f = psum.tile([128, 1], f32, tag="ps_f")
                ps_c = psum.tile([128, 1], f32, tag="ps_c")
                if n_pe:
                    ps_d = psum.tile([128, 1], f32, tag="ps_d")

                # ---- loads ----
                if idx_from == 'sbuf':
                    if labt_engine == 'sync':
                        nc.sync.dma_start(out=lab[:], in_=labt.ap())
                    else:
                        nc.gpsimd.dma_start(out=lab[:], in_=labt.ap())
                # Pool/SWDGE: feat with fp32->bf16 cast on the fly,
                # optionally in two pieces so compute can start earlier
                # (and the cs transfer can slot between them).
                if f_t_split > 0:
                    nc.gpsimd.dma_start(out=f_t[:, :f_t_split * D],
                                        in_=fv[:, :f_t_split, :])
                    nc.gpsimd.dma_start(out=f_t[:, f_t_split * D:],
                                        in_=fv[:, f_t_split:, :])
                else:
                    nc.gpsimd.dma_start(out=f_t[:], in_=fv[:, :, :])
                # HWDGE: center-slice (fp32)
                nc.sync.dma_start(out=cs_t[:], in_=csv[:, :, :])

                # Pool/SWDGE: one multi-index gather per chunk, bf16 out.
                # Index array read straight from DRAM in 'dram' mode: the
                # ExternalInput is staged in HBM before kernel start, so the
                # gather emission has no DMA dependency to wait on.
                for c in range(nch):
                    s0, s1 = bounds[c], bounds[c + 1]
                    idx_ap = (labt.ap()[:, s0:s1] if idx_from == 'dram'
                              else lab[:, s0:s1])
                    nc.gpsimd.indirect_dma_start(
                        out=cg[:, s0 * D:s1 * D],
                        out_offset=None,
                        in_=centers.ap(),
                        in_offset=bass.IndirectOffsetOnAxis(
                            ap=idx_ap, axis=0),
                    )

                # ---- elementwise reductions (3-way DVE/ACT/Pool split) ----
                # zero unused stats columns before any accum lands
                nc.vector.memset(stats[:], 0.0)

                # f^2: DVE part first (feat lands before cs in the DMA
                # queue), then ACT part split at the f_t_split boundary so
                # ACT can start on the first feat half early.
                if f2_dve_slots > 0:
                    sl = slice(0, f2_dve_slots * D)
                    nc.vector.scalar_tensor_tensor(
                        out=f_scr[:, sl], in0=f_t[:, sl], scalar=1.0,
                        in1=f_t[:, sl], op0=Alu.mult, op1=Alu.mult,
                        accum_out=stats[:, COL_F2:COL_F2 + 1])
                a0, a1 = f2_dve_slots, f2_dve_slots + f2_act_slots
                amid = min(max(f_t_split, a0), a1) if f_t_split > 0 else a1
                if amid > a0:
                    sl = slice(a0 * D, amid * D)
                    nc.scalar.activation(
                        out=f_scr[:, sl], in_=f_t[:, sl], func=Act.Square,
                        accum_out=stats[:, COL_F2 + 1:COL_F2 + 2])
                if a1 > amid:
                    sl = slice(amid * D, a1 * D)
                    nc.scalar.activation(
                        out=f_scr[:, sl], in_=f_t[:, sl], func=Act.Square,
                        accum_out=stats[:, COL_F2P:COL_F2P + 1])

                if cs2_engine == 'dve':
                    nc.vector.scalar_tensor_tensor(
                        out=cs_scr[:], in0=cs_t[:], scalar=1.0, in1=cs_t[:],
                        op0=Alu.mult, op1=Alu.mult,
                        accum_out=stats[:, COL_CS2:COL_CS2 + 1])
                else:
                    nc.scalar.activation(
                        out=cs_scr[:], in_=cs_t[:], func=Act.Square,
                        accum_out=stats[:, COL_CS2:COL_CS2 + 1])

                # per-chunk, per-piece: d = f - g (TT), then d^2 accum.
                # Each piece gets its own stats column so the dependency
                # graph stays piece-local.
                col = COL_D2
                pe_ranges = []
                for c in range(nch):
                    s0 = bounds[c]
                    # subs first (in piece order), then the d^2 ops, so an
                    # engine's d^2 for piece i never blocks a later sub.
                    ranges = []
                    p0 = s0
                    for sub_e, d2_e, ns in pieces[c]:
                        sl = slice(p0 * D, (p0 + ns) * D)
                        ranges.append((sl, d2_e))
                        eng = nc.vector if sub_e == 'dve' else nc.gpsimd
                        eng.tensor_tensor(
                            out=d_t[:, sl], in0=f_t[:, sl], in1=cg[:, sl],
                            op=Alu.subtract)
                        p0 += ns
                    for sl, d2_e in ranges:
                        if d2_e == 'dve':
                            nc.vector.scalar_tensor_tensor(
                                out=d_t[:, sl], in0=d_t[:, sl], scalar=1.0,
                                in1=d_t[:, sl], op0=Alu.mult, op1=Alu.mult,
                                accum_out=stats[:, col:col + 1])
                            col += 1
                        elif d2_e == 'act':
                            nc.scalar.activation(
                                out=cg[:, sl], in_=d_t[:, sl],
                                func=Act.Square,
                                accum_out=stats[:, col:col + 1])
                            col += 1
                        elif d2_e == 'actpe':
                            nc.scalar.activation(
                                out=cg[:, sl], in_=d_t[:, sl],
                                func=Act.Square)
                            pe_ranges.append(sl)
                        else:  # poolpe
                            nc.gpsimd.tensor_tensor(
                                out=cg[:, sl], in0=d_t[:, sl],
                                in1=d_t[:, sl], op=Alu.mult)
                            pe_ranges.append(sl)

                # ---- PE: column sums via [128,1]-output matmuls ----
                for j in range(CSPT):
                    nc.tensor.matmul(out=ps_c[:], lhsT=cs_t[:, j * D:(j + 1) * D],
                                     rhs=ones_f32[:], start=(j == 0),
                                     stop=(j == CSPT - 1))
                for j in range(NPT):
                    nc.tensor.matmul(out=ps_f[:], lhsT=f_t[:, j * D:(j + 1) * D],
                                     rhs=ones_bf[:], start=(j == 0),
                                     stop=(j == NPT - 1))
                # d^2 pe-reduced pieces: one matmul per 128-col slot into a
                # shared psum accumulate chain, then one copy to stats
                if pe_ranges:
                    blocks = []
                    for sl in pe_ranges:
                        for b in range(sl.start, sl.stop, D):
                            blocks.append(b)
                    for i, b in enumerate(blocks):
                        nc.tensor.matmul(out=ps_d[:], lhsT=cg[:, b:b + D],
                                         rhs=ones_bf[:], start=(i == 0),
                                         stop=(i == len(blocks) - 1))
                    nc.scalar.copy(stats[:, 13:14], ps_d[:])

                nc.scalar.copy(stats[:, COL_C:COL_C + 1], ps_c[:])
                nc.scalar.copy(stats[:, COL_F:COL_F + 1], ps_f[:])

                # ---- single output DMA ----
                nc.sync.dma_start(out=o_stats.ap(), in_=stats[:])

    nc.compile()
    return nc


def _get_nc(repeat=1, cfg=None):
    if cfg is None:
        cfg = CFG
    key = ("nc3", repeat, tuple(sorted(
        (k, tuple(v) if isinstance(v, (list, tuple)) else v)
        for k, v in cfg.items())))
    if key not in _cached:
        _cached[key] = _build_nc(repeat, cfg)
    return _cached[key]


def _make_in_maps(feat, label, centers):
    feat = np.ascontiguousarray(np.asarray(feat, dtype=np.float32))
    centers = np.ascontiguousarray(np.asarray(centers, dtype=np.float32))
    lab = np.asarray(label).astype(np.int32)
    in_maps = []
    for k in range(NCORES):
        fs = feat[k * BS:(k + 1) * BS]
        ls = lab[k * BS:(k + 1) * BS].reshape(128, NPT)
        cs = centers[k * CS:(k + 1) * CS]
        in_maps.append({
            "feat": np.ascontiguousarray(fs),
            "labt": np.ascontiguousarray(ls),
            "centers": centers,
            "cslice": np.ascontiguousarray(cs),
        })
    return in_maps


def _combine(results):
    sum_cs2 = 0.0
    sum_f2 = 0.0
    intra = 0.0
    F = np.zeros(D, dtype=np.float64)
    Cv = np.zeros(D, dtype=np.float64)
    for r in results:
        s = r["o_stats"].astype(np.float64)
        sum_cs2 += s[:, 0].sum()
        sum_f2 += s[:, 1:3].sum() + s[:, 12].sum()
        intra += s[:, 3:12].sum() + s[:, 13].sum()
        F += s[:, 14]
        Cv += s[:, 15]
    total = C * sum_f2 + B * sum_cs2 - 2.0 * float(F @ Cv)
    inter = total - intra
    loss = (LAMBDA_C / 2.0 / B) * intra / (inter + 1e-6) / 0.1
    return np.float32(loss)


def kernel(feat, label, centers):
    from concourse.bass_utils import run_bass_kernel_spmd

    nc = _get_nc()
    in_maps = _make_in_maps(feat, label, centers)
    res = run_bass_kernel_spmd(nc, in_maps, list(range(NCORES)))
    return _combine(res.results)


# revision 36
# speedup vs baseline: 1.0178x; 1.0164x over previous
"""ContrastiveCenterLoss on 8 Trainium2 NeuronCores.

Math: with dist[b,c] = ||f_b - c_c||^2,
  intra = sum_b dist[b, label_b]          = sum_b ||f_b - c_{label_b}||^2
  total = sum_{b,c} dist[b,c]             = C*sum||f||^2 + B*sum||c||^2 - 2*(sum_b f_b)@(sum_c c_c)
  inter = total - intra
  loss  = (1/2/B) * intra / (inter + 1e-6) / 0.1

Sharding: feat/label batch-sharded (2048 rows/core); centers statistics
sharded over 512-row slices; the full centers table stays in HBM and is
row-gathered by label via multi-index indirect DMA (bf16 cast on the way
in, 2 chunked SWDGE instructions).  intra is computed as sum (f-g)^2:
subtracts on DVE (bf16 TT at 2x), squares+accumulate distributed across
DVE (stt), ACT (Square activation), gpsimd (TT) and PE ([128,1]-output
ones-matmul reductions of squared scratch) per the tuned piece schedule
in CFG.  feat loads as two SWDGE bf16 cast-DMAs so f^2 and the column
sums start early; the ACT Square table is warmed during the load phase.
Per core the kernel emits a single [128, 16] fp32 stats tile:
per-partition partial sums of cs^2, f^2, (f-g)^2 plus the per-d column
sums F = sum_b f_b and Cs = sum_c c_c (from [128,1]-output PE matmuls).
Host all-reduces the per-core partials in float64 and applies the final
scalar division.
"""

import numpy as np

B, C, D = 16384, 4096, 128
LAMBDA_C = 1.0
NCORES = 8
BS = B // NCORES          # 2048 feat rows per core
NPT = BS // 128           # 16 feat rows per partition
CS = C // NCORES          # 512 center rows per core (stats slice)
CSPT = CS // 128          # 4 center rows per partition

# tuned via TimelineSim search (see test.py / search harness)
CFG = dict(
    chunks=(10, 16),      # gather chunk boundaries in slot units
    # per chunk: list of (sub_engine, d2_engine, nslots) pieces.
    # sub_engine: 'dve' (TT bf16 2x) or 'pool' (gpsimd TT).
    # d2_engine: 'dve' (stt accum) or 'act' (Square accum).
    # tuned via TimelineSim search (search.py / search3.py / search4.py)
    pieces=(
        (('dve', 'actpe', 4), ('dve', 'act', 6)),
        (('dve', 'poolpe', 2), ('dve', 'dve', 4)),
    ),
    ps_d_copy='dve',      # PE-reduce psum -> stats copy engine
    f2_dve_slots=4,       # slots of f^2 on DVE (rest ACT)
    f2_act_slots=12,
    labt_engine='sync',   # 'sync' (HWDGE) or 'gpsimd' (SWDGE)
    cs2_engine='dve',     # 'dve' | 'act'
    f_t_split=8,          # slots in first feat DMA (0 = single DMA)
    idx_from='sbuf',      # 'sbuf' (DMA labels in first) or 'dram' (direct)
)

_cached = {}


def _build_nc(repeat=1, cfg=None):
    import concourse.bass as bass
    import concourse.tile as tile
    from concourse import bacc, mybir

    if cfg is None:
        cfg = CFG
    chunks = cfg['chunks']
    pieces = cfg['pieces']
    sizes = [b - a for a, b in zip((0,) + tuple(chunks[:-1]), chunks)]
    f2_dve_slots = cfg['f2_dve_slots']
    f2_act_slots = cfg.get('f2_act_slots', NPT - cfg['f2_dve_slots'])
    labt_engine = cfg['labt_engine']
    cs2_engine = cfg['cs2_engine']
    idx_from = cfg.get('idx_from', 'sbuf')
    f_t_split = cfg.get('f_t_split', 0)   # slots in first feat DMA (0=one)
    assert len(pieces) == len(chunks)
    for s, pl in zip(sizes, pieces):
        assert sum(p[2] for p in pl) == s, (sizes, pieces)
        # 'dve'/'act' accumulate directly; 'actpe'/'poolpe' square without
        # an accumulator and reduce via [128,1]-output PE matmuls
        assert all(p[1] in ('dve', 'act', 'actpe', 'poolpe') for p in pl)
        assert all(p[0] in ('dve', 'pool') for p in pl)
    n_pe = sum(p[2] for pl in pieces for p in pl if p[1] in ('actpe', 'poolpe'))
    n_pieces = sum(len(pl) for pl in pieces)
    assert n_pieces <= 9, "not enough stats columns (3..11 for d^2)"
    assert f2_dve_slots + f2_act_slots <= NPT

    f32 = mybir.dt.float32
    bf16 = mybir.dt.bfloat16
    i32 = mybir.dt.int32
    Alu = mybir.AluOpType
    Act = mybir.ActivationFunctionType

    nc = bacc.Bacc("TRN2", target_bir_lowering=False, debug=False,
                   num_devices=NCORES)

    feat = nc.dram_tensor("feat", [BS, D], f32, kind="ExternalInput")
    labt = nc.dram_tensor("labt", [128, NPT], i32, kind="ExternalInput")
    centers = nc.dram_tensor("centers", [C, D], f32, kind="ExternalInput")
    cslice = nc.dram_tensor("cslice", [CS, D], f32, kind="ExternalInput")

    o_stats = nc.dram_tensor("o_stats", [128, 16], f32, kind="ExternalOutput")

    # stats column map; host sums cols 3..13 as intra, 1..2 (+13) as f^2
    COL_CS2 = 0
    COL_F2 = 1            # 1 dve, 2 act
    COL_D2 = 3            # 3 + chunk (DVE), 6 + chunk (ACT), 9 + chunk (Pool)
    COL_D2A = 6
    COL_D2P = 9
    COL_F2P = 12          # f^2 pool part
    COL_F = 14
    COL_C = 15

    # partition-major views: partition p holds rows p*NPT .. p*NPT+NPT-1
    fv = feat.ap().rearrange("(p n) d -> p n d", p=128)
    csv = cslice.ap().rearrange("(p n) d -> p n d", p=128)

    bounds = [0] + list(chunks)
    nch = len(chunks)

    with tile.TileContext(nc) as tc:
        with tc.tile_pool(name="const", bufs=1) as cpool, \
             tc.tile_pool(name="sbuf", bufs=2) as pool, \
             tc.tile_pool(name="psum", bufs=2, space="PSUM") as psum:

            ones_bf = cpool.tile([128, 1], bf16)
            nc.vector.memset(ones_bf[:], 1.0)
            ones_f32 = cpool.tile([128, 1], f32)
            nc.vector.memset(ones_f32[:], 1.0)
            # warm the ACT Square table during the load phase: the table
            # load costs ~1.3us and otherwise lands right before the first
            # real Square, stalling the ACT stream mid-kernel.
            warm = cpool.tile([128, 1], f32)
            nc.scalar.activation(
                out=warm[:], in_=ones_f32[:],
                func=mybir.ActivationFunctionType.Square)

            for _ in range(repeat):
                lab = pool.tile([128, NPT], i32, tag="lab")
                f_t = pool.tile([128, NPT * D], bf16, tag="f_t")
                cg = pool.tile([128, NPT * D], bf16, tag="cg")
                d_t = pool.tile([128, NPT * D], bf16, tag="d_t")
                cs_t = pool.tile([128, CSPT * D], f32, tag="cs_t")
                cs_scr = pool.tile([128, CSPT * D], f32, tag="cs_scr")
                f_scr = pool.tile([128, NPT * D], bf16, tag="f_scr")
                stats = pool.tile([128, 16], f32, tag="stats")

                ps_f = psum.tile([128, 1], f32, tag="ps_f")
                ps_c = psum.tile([128, 1], f32, tag="ps_c")
                if n_pe:
                    ps_d = psum.tile([128, 1], f32, tag="ps_d")

                # ---- loads ----
                if idx_from == 'sbuf':
                    if labt_engine == 'sync':
                        nc.sync.dma_start(out=lab[:], in_=labt.ap())
                    else:
                        nc.gpsimd.dma_start(out=lab[:], in_=labt.ap())
                # Pool/SWDGE: feat with fp32->bf16 cast on the fly,
                # optionally in two pieces so compute can start earlier
                # (and the cs transfer can slot between them).
                if f_t_split > 0:
                    nc.gpsimd.dma_start(out=f_t[:, :f_t_split * D],
                                        in_=fv[:, :f_t_split, :])
                    nc.gpsimd.dma_start(out=f_t[:, f_t_split * D:],
                                        in_=fv[:, f_t_split:, :])
                else:
                    nc.gpsimd.dma_start(out=f_t[:], in_=fv[:, :, :])
                # HWDGE: center-slice (fp32)
                nc.sync.dma_start(out=cs_t[:], in_=csv[:, :, :])

                # Pool/SWDGE: one multi-index gather per chunk, bf16 out.
                # Index array read straight from DRAM in 'dram' mode: the
                # ExternalInput is staged in HBM before kernel start, so the
                # gather emission has no DMA dependency to wait on.
                for c in range(nch):
                    s0, s1 = bounds[c], bounds[c + 1]
                    idx_ap = (labt.ap()[:, s0:s1] if idx_from == 'dram'
                              else lab[:, s0:s1])
                    nc.gpsimd.indirect_dma_start(
                        out=cg[:, s0 * D:s1 * D],
                        out_offset=None,
                        in_=centers.ap(),
                        in_offset=bass.IndirectOffsetOnAxis(
                            ap=idx_ap, axis=0),
                    )

                # ---- elementwise reductions (3-way DVE/ACT/Pool split) ----
                # zero unused stats columns before any accum lands
                nc.vector.memset(stats[:], 0.0)

                # f^2: DVE part first (feat lands before cs in the DMA
                # queue), then ACT part split at the f_t_split boundary so
                # ACT can start on the first feat half early.
                if f2_dve_slots > 0:
                    sl = slice(0, f2_dve_slots * D)
                    nc.vector.scalar_tensor_tensor(
                        out=f_scr[:, sl], in0=f_t[:, sl], scalar=1.0,
                        in1=f_t[:, sl], op0=Alu.mult, op1=Alu.mult,
                        accum_out=stats[:, COL_F2:COL_F2 + 1])
                a0, a1 = f2_dve_slots, f2_dve_slots + f2_act_slots
                amid = min(max(f_t_split, a0), a1) if f_t_split > 0 else a1
                if amid > a0:
                    sl = slice(a0 * D, amid * D)
                    nc.scalar.activation(
                        out=f_scr[:, sl], in_=f_t[:, sl], func=Act.Square,
                        accum_out=stats[:, COL_F2 + 1:COL_F2 + 2])
                if a1 > amid:
                    sl = slice(amid * D, a1 * D)
                    nc.scalar.activation(
                        out=f_scr[:, sl], in_=f_t[:, sl], func=Act.Square,
                        accum_out=stats[:, COL_F2P:COL_F2P + 1])

                if cs2_engine == 'dve':
                    nc.vector.scalar_tensor_tensor(
                        out=cs_scr[:], in0=cs_t[:], scalar=1.0, in1=cs_t[:],
                        op0=Alu.mult, op1=Alu.mult,
                        accum_out=stats[:, COL_CS2:COL_CS2 + 1])
                else:
                    nc.scalar.activation(
                        out=cs_scr[:], in_=cs_t[:], func=Act.Square,
                        accum_out=stats[:, COL_CS2:COL_CS2 + 1])

                # per-chunk, per-piece: d = f - g (TT), then d^2 accum.
                # Each piece gets its own stats column so the dependency
                # graph stays piece-local.
                col = COL_D2
                pe_ranges = []
                for c in range(nch):
                    s0 = bounds[c]
                    # subs first (in piece order), then the d^2 ops, so an
                    # engine's d^2 for piece i never blocks a later sub.
                    ranges = []
                    p0 = s0
                    for sub_e, d2_e, ns in pieces[c]:
                        sl = slice(p0 * D, (p0 + ns) * D)
                        ranges.append((sl, d2_e))
                        eng = nc.vector if sub_e == 'dve' else nc.gpsimd
                        eng.tensor_tensor(
                            out=d_t[:, sl], in0=f_t[:, sl], in1=cg[:, sl],
                            op=Alu.subtract)
                        p0 += ns
                    for sl, d2_e in ranges:
                        if d2_e == 'dve':
                            nc.vector.scalar_tensor_tensor(
                                out=d_t[:, sl], in0=d_t[:, sl], scalar=1.0,
                                in1=d_t[:, sl], op0=Alu.mult, op1=Alu.mult,
                                accum_out=stats[:, col:col + 1])
                            col += 1
                        elif d2_e == 'act':
                            nc.scalar.activation(
                                out=cg[:, sl], in_=d_t[:, sl],
                                func=Act.Square,
                                accum_out=stats[:, col:col + 1])
                            col += 1
                        elif d2_e == 'actpe':
                            nc.scalar.activation(
                                out=cg[:, sl], in_=d_t[:, sl],
                                func=Act.Square)
                            pe_ranges.append(sl)
                        else:  # poolpe
                            nc.gpsimd.tensor_tensor(
                                out=cg[:, sl], in0=d_t[:, sl],
                                in1=d_t[:, sl], op=Alu.mult)
                            pe_ranges.append(sl)

                # ---- PE: column sums via [128,1]-output matmuls ----
                for j in range(CSPT):
                    nc.tensor.matmul(out=ps_c[:], lhsT=cs_t[:, j * D:(j + 1) * D],
                                     rhs=ones_f32[:], start=(j == 0),
                                     stop=(j == CSPT - 1))
                for j in range(NPT):
                    nc.tensor.matmul(out=ps_f[:], lhsT=f_t[:, j * D:(j + 1) * D],
                                     rhs=ones_bf[:], start=(j == 0),
                                     stop=(j == NPT - 1))
                # d^2 pe-reduced pieces: one matmul per 128-col slot into a
                # shared psum accumulate chain, then one copy to stats
                if pe_ranges:
                    blocks = []
                    for sl in pe_ranges:
                        for b in range(sl.start, sl.stop, D):
                            blocks.append(b)
                    for i, b in enumerate(blocks):
                        nc.tensor.matmul(out=ps_d[:], lhsT=cg[:, b:b + D],
                                         rhs=ones_bf[:], start=(i == 0),
                                         stop=(i == len(blocks) - 1))
                    if cfg.get('ps_d_copy', 'act') == 'dve':
                        nc.vector.tensor_copy(out=stats[:, 13:14],
                                              in_=ps_d[:])
                    else:
                        nc.scalar.copy(stats[:, 13:14], ps_d[:])

                nc.scalar.copy(stats[:, COL_C:COL_C + 1], ps_c[:])
                nc.scalar.copy(stats[:, COL_F:COL_F + 1], ps_f[:])

                # ---- single output DMA ----
                nc.sync.dma_start(out=o_stats.ap(), in_=stats[:])

    nc.compile()
    return nc


def _get_nc(repeat=1, cfg=None):
    if cfg is None:
        cfg = CFG
    key = ("nc3", repeat, tuple(sorted(
        (k, tuple(v) if isinstance(v, (list, tuple)) else v)
        for k, v in cfg.items())))
    if key not in _cached:
        _cached[key] = _build_nc(repeat, cfg)
    return _cached[key]


def _make_in_maps(feat, label, centers):
    feat = np.ascontiguousarray(np.asarray(feat, dtype=np.float32))
    centers = np.ascontiguousarray(np.asarray(centers, dtype=np.float32))
    lab = np.asarray(label).astype(np.int32)
    in_maps = []
    for k in range(NCORES):
        fs = feat[k * BS:(k + 1) * BS]
        ls = lab[k * BS:(k + 1) * BS].reshape(128, NPT)
        cs = centers[k * CS:(k + 1) * CS]
        in_maps.append({
            "feat": np.ascontiguousarray(fs),
            "labt": np.ascontiguousarray(ls),
            "centers": centers,
            "cslice": np.ascontiguousarray(cs),
        })
    return in_maps


def _combine(results):
    sum_cs2 = 0.0
    sum_f2 = 0.0
    intra = 0.0
    F = np.zeros(D, dtype=np.float64)
    Cv = np.zeros(D, dtype=np.float64)
    for r in results:
        s = r["o_stats"].astype(np.float64)
        sum_cs2 += s[:, 0].sum()
        sum_f2 += s[:, 1:3].sum() + s[:, 12].sum()
        intra += s[:, 3:12].sum() + s[:, 13].sum()
        F += s[:, 14]
        Cv += s[:, 15]
    total = C * sum_f2 + B * sum_cs2 - 2.0 * float(F @ Cv)
    inter = total - intra
    loss = (LAMBDA_C / 2.0 / B) * intra / (inter + 1e-6) / 0.1
    return np.float32(loss)


def kernel(feat, label, centers):
    from concourse.bass_utils import run_bass_kernel_spmd

    nc = _get_nc()
    in_maps = _make_in_maps(feat, label, centers)
    res = run_bass_kernel_spmd(nc, in_maps, list(range(NCORES)))
    return _combine(res.results)
